# revision 42
# baseline (speedup 1.0000x reference)
"""Trainium2 Bass kernel for nn_ContrastiveLoss (SCAN text-to-image loss).

Full inputs in, full (scalar) output out. Captions are length-balanced
(LPT) across the 8 NeuronCores and their valid words RAGGED-PACKED into
CWP columns (232 for the reference input, vs 16*24=384 padded); images
are replicated. Each core computes scores^T [16, 128] for its caption
slice, an AllGather assembles the (caption-permuted) [C, I] score matrix,
and every core computes the diagonal-margin loss redundantly with
permutation-aware eye masks; core 0's value is returned.

Math notes (exact reductions of the reference):
  - softmax over regions needs no normalizer: with E = exp(9 * a_norm),
    cos = (sum_r E*A) / (||cap|| * sqrt(E^T G E)) since the softmax
    normalizer Z cancels between numerator and denominator.
  - wei-norm uses the per-image Gram matrix G_i = X_i X_i^T, so the
    [C,I,W,D] weiContext tensor is never materialized.
  - ragged per-caption word reductions (the l2norm over words and the
    final logsumexp) contract over PACKED word columns via a PE
    transpose + host-built 0/1 selector matmul; the reverse broadcast
    (per-caption rinv back to packed columns) is a 0/1 expander matmul.
  - 1/sqrt(x) is exp(-0.5*ln(x)) so every scalar-engine function lives
    in one activation table (no 1.3us table reloads).
  - all matmul operands are bf16 (fp32 pays 4 cycles/row; f32r pays it
    below 256 columns); PSUM accumulation stays fp32.

Scheduling: one flat software pipeline over (rep, group). The PE is
in-order with a 4-deep wait queue, so stages are emitted so that every
instruction's cross-engine producer finished a full iteration earlier;
each rep's epilogue + AllGather + loss hides under the next rep's
group compute.
"""

import numpy as np

# Problem geometry (hardcoded per contract).
I, R, D, W = 128, 36, 512, 24
NCORES = 8
CS = I // NCORES          # captions per core = 16
GI = 3                    # images per PE group (3*36 = 108 <= 128 partitions)
GR = GI * R               # 108
NG = (I + GI - 1) // GI   # 43 groups
IRP = NG * GR             # 4644 padded image-region columns
NK = D // 128             # 4 contraction chunks

_CACHE: dict = {}


def _bf16():
    import concourse.mybir as mybir
    return mybir.dt.np(mybir.dt.bfloat16)


def _plan(cap_lens):
    """LPT-balance the 128 captions into 8 bins of 16; return bins and the
    packed word-column budget CWP (max bin load, padded to a multiple of 8,
    clamped into (128, 256] for the two-chunk transpose layout)."""
    cl = np.asarray(cap_lens)
    order = np.argsort(-cl, kind="stable")
    bins = [[] for _ in range(NCORES)]
    loads = [0] * NCORES
    for c in order:
        cands = [b for b in range(NCORES) if len(bins[b]) < CS]
        b = min(cands, key=lambda bb: (loads[bb], bb))
        bins[b].append(int(c))
        loads[b] += int(cl[c])
    cwp = max(136, -(-max(loads) // 8) * 8)
    assert cwp <= 256, (
        f"packed word budget {cwp} > 256 unsupported (caption lengths sum "
        f"too large for the 2-chunk layout)"
    )
    return bins, cwp


def _build_program(cwp: int, reps: int = 1, with_loss: bool = True):
    import concourse.bacc as bacc
    import concourse.mybir as mybir
    import concourse.tile as tile

    f32 = mybir.dt.float32
    bf16 = mybir.dt.bfloat16
    Act = mybir.ActivationFunctionType
    Alu = mybir.AluOpType
    X = mybir.AxisListType.X

    CWP = cwp
    CH = [0, CWP - 128]       # transpose chunk column starts
    GC = GR + CWP             # per-group column block [imT_g | capT]
    H = reps * NG             # global group count

    nc = bacc.Bacc("TRN2", target_bir_lowering=False, debug=False,
                   num_devices=NCORES)

    imcap_d = nc.dram_tensor("imcap", [NK, 128, NG * GC], bf16,
                             kind="ExternalInput")
    sel_d = nc.dram_tensor("sel", [128, 2 * CS], bf16, kind="ExternalInput")
    exp_d = nc.dram_tensor("expand", [CS, CWP], bf16, kind="ExternalInput")
    inds_d = nc.dram_tensor("inds", [GR, NG * 128], bf16,
                            kind="ExternalInput")
    bmask_d = nc.dram_tensor("bmask", [GR, GR], f32, kind="ExternalInput")
    padqc_d = nc.dram_tensor("padqc", [128, CWP], f32, kind="ExternalInput")
    onesb_d = nc.dram_tensor("onesb", [128, 128], bf16, kind="ExternalInput")
    eyeb_d = nc.dram_tensor("eyeb", [128, 128], bf16, kind="ExternalInput")
    eyef_d = nc.dram_tensor("eyef", [128, 128], f32, kind="ExternalInput")
    eyeP_d = nc.dram_tensor("eyeP", [128, 128], f32, kind="ExternalInput")
    neyeP_d = nc.dram_tensor("neyeP", [128, 128], f32, kind="ExternalInput")
    eyePT_d = nc.dram_tensor("eyePT", [128, 128], f32, kind="ExternalInput")
    neyePT_d = nc.dram_tensor("neyePT", [128, 128], f32,
                              kind="ExternalInput")
    onescol_d = nc.dram_tensor("onescol", [128, 1], f32,
                               kind="ExternalInput")
    loss_d = nc.dram_tensor("loss", [1, 1], f32, kind="ExternalOutput")

    with tile.TileContext(nc) as tc:
        with (
            tc.tile_pool(name="const", bufs=1) as cp,
            tc.tile_pool(name="work", bufs=6) as wp,
            tc.tile_pool(name="small", bufs=7) as sp,
            tc.tile_pool(name="fin", bufs=2) as fe,
            tc.tile_pool(name="dram", bufs=1, space="DRAM") as dp,
            tc.tile_pool(name="ps_acc", bufs=1, space="PSUM") as pa,
            tc.tile_pool(name="ps_ag", bufs=4, space="PSUM") as pag,
            tc.tile_pool(name="ps_re", bufs=2, space="PSUM") as prege,
            tc.tile_pool(name="ps_fin", bufs=1, space="PSUM") as pf,
        ):
            # ---- constants -------------------------------------------------
            sel = cp.tile([128, 2 * CS], bf16, tag="sel")
            expand = cp.tile([CS, CWP], bf16, tag="expand")
            inds = cp.tile([GR, NG * 128], bf16, tag="inds")
            bmask = cp.tile([GR, GR], f32, tag="bmask")
            padqc = cp.tile([128, CWP], f32, tag="padqc")
            onesb = cp.tile([128, 128], bf16, tag="onesb")
            eyeb = cp.tile([128, 128], bf16, tag="eyeb")
            eyef = cp.tile([128, 128], f32, tag="eyef")
            eyeP = cp.tile([128, 128], f32, tag="eyeP")
            neyeP = cp.tile([128, 128], f32, tag="neyeP")
            eyePT = cp.tile([128, 128], f32, tag="eyePT")
            neyePT = cp.tile([128, 128], f32, tag="neyePT")
            onescol = cp.tile([128, 1], f32, tag="onescol")
            scoresT = cp.tile([CS, 128], f32, tag="scoresT")
            sfull = cp.tile([128, 128], f32, tag="sfull")
            imcap = [cp.tile([128, NG * GC], bf16, tag=f"imcap{k}",
                             name=f"imcap{k}") for k in range(NK)]

            nc.sync.dma_start(out=sel[:], in_=sel_d[:])
            nc.sync.dma_start(out=expand[:], in_=exp_d[:])
            nc.sync.dma_start(out=inds[:], in_=inds_d[:])
            nc.sync.dma_start(out=bmask[:], in_=bmask_d[:])
            nc.sync.dma_start(out=padqc[:], in_=padqc_d[:])
            nc.sync.dma_start(out=onesb[:], in_=onesb_d[:])
            nc.sync.dma_start(out=eyeb[:], in_=eyeb_d[:])
            nc.sync.dma_start(out=eyef[:], in_=eyef_d[:])
            nc.sync.dma_start(out=eyeP[:], in_=eyeP_d[:])
            nc.sync.dma_start(out=neyeP[:], in_=neyeP_d[:])
            nc.sync.dma_start(out=eyePT[:], in_=eyePT_d[:])
            nc.sync.dma_start(out=neyePT[:], in_=neyePT_d[:])
            nc.sync.dma_start(out=onescol[:], in_=onescol_d[:])
            # [imT_g | capT] blocks resident in SBUF; column-block-major DMA
            # order so early groups get all 4 contraction chunks first
            CB = 4 * GC
            for c0 in range(0, NG * GC, CB):
                c1 = min(c0 + CB, NG * GC)
                for k in range(NK):
                    nc.sync.dma_start(out=imcap[k][:, c0:c1],
                                      in_=imcap_d[k, :, c0:c1])

            # Persistent PSUM layout (8 banks total):
            #   pa:  ps_neq [128, 2*CWP]            (1 bank)
            #   pag: AG [gram|attn|bt/n2t] x4       (4 banks)
            #   prege: REGE [rinvE|GE] x2           (2 banks)
            #   pf:  PFB shared epilogue bank       (1 bank):
            #        cn2ps [128, 0:CWP] / ps_t [128, 0:128] (tail reuse)
            #        ext   bf16 cols [2*CWP : 2*CWP+256]
            #        rsT   [16, CWP+128 : CWP+256]
            #        ps_l  [1, CWP+256]
            ps_neq = pa.tile([128, 2 * CWP], f32, tag="neq", name="ps_neq")
            PFB = pf.tile([128, 512], f32, tag="pfb", name="PFB")
            cn2ps = PFB[:, 0:CWP]
            PFBb = PFB[:].bitcast(bf16)
            ext = PFBb[:, 2 * CWP:2 * CWP + 256]
            rsTps = PFB[0:CS, CWP + 128:CWP + 256]
            ps_l = PFB[0:1, CWP + 256:CWP + 257]
            ps_t = PFB[:, 0:128]

            st: dict = {}
            cn2_of: dict = {}
            bto = 2 * (GR + CWP)

            # ---- per-group pipeline stages ---------------------------------
            def s0(h):
                # Bank AG packs [gram(108) | attn(CWP) | bt/n2t]. One matmul
                # per k chunk computes [gram | attn] together: the moving
                # operand is the host-interleaved [imT_g | capT] block and
                # the stationary is imT_g.
                AG = pag.tile([128, 512], f32, tag="ag", name="AG")
                st[h] = {"AG": AG}
                g = h % NG
                for k in range(NK):
                    blk = imcap[k][:, g * GC:(g + 1) * GC]
                    sl = imcap[k][:, g * GC:g * GC + GR]
                    nc.tensor.matmul(AG[0:GR, 0:GC], sl, blk,
                                     start=(k == 0), stop=(k == NK - 1))

            def s1_act(h):
                B = wp.tile([GR, CWP], bf16, tag="B", name="B")
                st[h]["B"] = B
                nc.scalar.activation(B[:], st[h]["AG"][0:GR, GR:GR + CWP],
                                     Act.Prelu, alpha=0.1)

            def s1_dve(h):
                g_sb = sp.tile([GR, GR], bf16, tag="gsb", name="g_sb")
                st[h]["g_sb"] = g_sb
                nc.vector.tensor_tensor(g_sb[:], st[h]["AG"][0:GR, 0:GR],
                                        bmask[:], Alu.mult)

            def s2_dve(h):
                # odd groups: square in SBUF first (DVE), so the
                # post-transpose pass is a cheap copy
                B2 = wp.tile([GR, CWP], bf16, tag="B2sq", name="B2")
                st[h]["B2"] = B2
                nc.vector.tensor_tensor(B2[:], st[h]["B"][:], st[h]["B"][:],
                                        Alu.mult)

            def s2(h):
                # transpose B (or B^2) as 2 overlapping 128-col chunks so the
                # ragged per-caption word reduction becomes a matmul; lands
                # in AG's spare bytes as bf16
                AGb = st[h]["AG"][:].bitcast(bf16)
                src = st[h]["B2"] if h % 2 else st[h]["B"]
                for p, c0 in enumerate(CH):
                    nc.tensor.transpose(
                        AGb[:, bto + p * GR:bto + (p + 1) * GR],
                        src[:, c0:c0 + 128], eyeb[0:GR, 0:GR])

            def s_a2(h):
                # A2 = inverse-leaky(B) = min(B, 10B); the 10x runs on the
                # otherwise-idle Pool engine (supports mult but not min)
                t10 = wp.tile([GR, CWP], bf16, tag="t10", name="t10")
                st[h]["t10"] = t10
                nc.gpsimd.tensor_scalar_mul(t10[:], st[h]["B"][:], 10.0)

            def s_a2b(h):
                A2 = wp.tile([GR, CWP], bf16, tag="A2", name="A2")
                st[h]["A2"] = A2
                nc.vector.tensor_tensor(A2[:], st[h]["t10"][:],
                                        st[h]["B"][:], Alu.min)

            def s3(h):
                # PSUM -> SBUF pass: Act square (even) or DVE copy (odd);
                # DVE cannot square PSUM in place (one-PSUM-operand rule)
                AGb = st[h]["AG"][:].bitcast(bf16)
                B2t = wp.tile([128, 2 * GR], bf16, tag="B2t", name="B2t")
                st[h]["B2t"] = B2t
                if h % 2:
                    nc.vector.tensor_scalar(B2t[:],
                                            AGb[:, bto:bto + 2 * GR],
                                            0.0, None, Alu.add)
                else:
                    nc.scalar.activation(B2t[:], AGb[:, bto:bto + 2 * GR],
                                         Act.Square)

            def s4(h):
                # n2t[s, gr] = sum_j sel[j,s] * Bt[j,gr]^2
                # (reuses the bt bytes -- bt is dead after B2t)
                n2t = st[h]["AG"][0:CS, GR + CWP:GR + CWP + GR]
                B2t = st[h]["B2t"]
                for p in range(2):
                    nc.tensor.matmul(n2t, sel[:, p * CS:(p + 1) * CS],
                                     B2t[:, p * GR:(p + 1) * GR],
                                     start=(p == 0), stop=(p == 1))

            def s5(h):
                n2t = st[h]["AG"][0:CS, GR + CWP:GR + CWP + GR]
                lnn = sp.tile([CS, GR], f32, tag="lnn", name="lnn")
                nc.scalar.activation(lnn[:], n2t, Act.Ln)
                rinvT = sp.tile([CS, GR], bf16, tag="rinvT", name="rinvT")
                st[h]["rinvT"] = rinvT
                nc.scalar.activation(rinvT[:], lnn[:], Act.Exp, scale=-0.5)

            def s6(h):
                # Bank REGE packs [rinvE(CWP) | GE(CWP)]
                REGE = prege.tile([GR, 2 * CWP], f32, tag="rege",
                                  name="REGE")
                st[h]["REGE"] = REGE
                nc.tensor.matmul(REGE[:, 0:CWP], st[h]["rinvT"][:],
                                 expand[:], start=True, stop=True)

            def s7_dve(h):
                Bn = wp.tile([GR, CWP], f32, tag="Bn", name="Bn")
                st[h]["Bn"] = Bn
                nc.vector.tensor_tensor(Bn[:], st[h]["B"][:],
                                        st[h]["REGE"][:, 0:CWP], Alu.mult)

            def s7_act(h):
                E = wp.tile([GR, CWP], bf16, tag="E", name="E")
                st[h]["E"] = E
                nc.scalar.activation(E[:], st[h]["Bn"][:], Act.Exp,
                                     scale=9.0)

            def s7_pa(h):
                pair = wp.tile([GR, 2 * CWP], bf16, tag="pair", name="pair")
                st[h]["pair"] = pair
                nc.vector.tensor_tensor(pair[:, 0:CWP], st[h]["E"][:],
                                        st[h]["A2"][:], Alu.mult)

            def s8(h):
                nc.tensor.matmul(st[h]["REGE"][:, CWP:2 * CWP],
                                 st[h]["g_sb"][:], st[h]["E"][:],
                                 start=True, stop=True)

            def s9(h):
                nc.vector.tensor_tensor(st[h]["pair"][:, CWP:2 * CWP],
                                        st[h]["E"][:],
                                        st[h]["REGE"][:, CWP:2 * CWP],
                                        Alu.mult)

            def s10(h):
                g = h % NG
                nc.tensor.matmul(ps_neq[:],
                                 inds[:, g * 128:(g + 1) * 128],
                                 st[h]["pair"][:],
                                 start=(g == 0), stop=(g == NG - 1))
                del st[h]

            # ---- per-rep boundary stages -----------------------------------
            def emit_cn2(rep):
                # caption word-norms^2, replicated over partitions
                for k in range(NK):
                    capk = imcap[k][:, GR:GC]         # capT copy in group 0
                    sq = wp.tile([128, CWP], bf16, tag="csq", name="sq")
                    nc.vector.tensor_tensor(sq[:], capk, capk, Alu.mult)
                    nc.tensor.matmul(cn2ps, onesb[:], sq[:],
                                     start=(k == 0), stop=(k == NK - 1))
                cn2 = fe.tile([128, CWP], f32, tag="cn2", name="cn2")
                cn2_of[rep] = cn2
                nc.scalar.copy(cn2[:], cn2ps)

            head_ex: dict = {}

            def emit_head_a(rep):
                # epilogue part 1 (DVE/Act only): cos -> exp(6 cos)
                cn2 = cn2_of.pop(rep)
                qc = fe.tile([128, CWP], f32, tag="qc", name="qc")
                nc.vector.tensor_tensor(qc[:], ps_neq[:, CWP:2 * CWP],
                                        cn2[:], Alu.mult)
                qc2 = fe.tile([128, CWP], f32, tag="qc2", name="qc2")
                nc.vector.tensor_tensor(qc2[:], qc[:], padqc[:], Alu.add)
                lq = fe.tile([128, CWP], f32, tag="lq", name="lq")
                nc.scalar.activation(lq[:], qc2[:], Act.Ln)
                rsq = fe.tile([128, CWP], f32, tag="rsq", name="rsq")
                nc.scalar.activation(rsq[:], lq[:], Act.Exp, scale=-0.5)
                cosm = fe.tile([128, CWP], f32, tag="cosm", name="cosm")
                nc.vector.tensor_tensor(cosm[:], ps_neq[:, 0:CWP], rsq[:],
                                        Alu.mult)
                ex = fe.tile([128, CWP], bf16, tag="ex", name="ex")
                head_ex[rep] = ex
                nc.scalar.activation(ex[:], cosm[:], Act.Exp, scale=6.0)

            def emit_head_b(rep):
                # epilogue part 2 (PE-heavy, two iterations later so the
                # Act chain above is done): ragged per-caption sum via
                # transpose + selector, then scores^T and the AllGather
                ex = head_ex.pop(rep)
                for p, c0 in enumerate(CH):
                    nc.tensor.transpose(ext[:, p * 128:(p + 1) * 128],
                                        ex[:, c0:c0 + 128], eyeb[:])
                exts = fe.tile([128, 256], bf16, tag="exts", name="exts")
                nc.scalar.copy(exts[:], ext[:])
                for p in range(2):
                    nc.tensor.matmul(rsTps, sel[:, p * CS:(p + 1) * CS],
                                     exts[:, p * 128:(p + 1) * 128],
                                     start=(p == 0), stop=(p == 1))
                # scoresT (x6): ln(sum) = 6 * row_sim, caption-major
                nc.scalar.activation(scoresT[:], rsTps, Act.Ln)

                if not with_loss:
                    if rep == reps - 1:
                        nc.sync.dma_start(out=loss_d[:],
                                          in_=scoresT[0:1, 0:1])
                    return
                sl_dram = dp.tile([CS, 128], f32, tag="sl", name="sl_dram")
                ag_dram = dp.tile([NCORES, CS, 128], f32, tag="ag",
                                  name="ag_dram")
                nc.sync.dma_start(out=sl_dram[:], in_=scoresT[:])
                nc.gpsimd.collective_compute(
                    "AllGather", Alu.bypass,
                    replica_groups=[list(range(NCORES))],
                    ins=[sl_dram.opt()], outs=[ag_dram.opt()],
                )
                nc.sync.dma_start(
                    out=sfull[:],
                    in_=ag_dram[:].rearrange("r p i -> (r p) i"))

            def emit_tail(rep):
                # diagonal-margin loss on the gathered 6*scores^T matrix:
                # rows p = permuted captions, cols = images
                de = fe.tile([128, 128], f32, tag="de", name="de")
                nc.vector.tensor_tensor(de[:], sfull[:], eyeP[:], Alu.mult)
                diag = fe.tile([128, 1], f32, tag="diag", name="diag")
                nc.vector.reduce_sum(diag[:], de[:], axis=X)
                dm = fe.tile([128, 1], f32, tag="dm", name="dm")
                nc.vector.tensor_scalar(dm[:], diag[:], 1.2, None,
                                        Alu.subtract)
                m1 = fe.tile([128, 128], f32, tag="m1", name="m1")
                nc.vector.tensor_scalar(m1[:], sfull[:], dm[:], 0.0,
                                        Alu.subtract, Alu.max)
                m1e = fe.tile([128, 128], f32, tag="m1e", name="m1e")
                nc.vector.tensor_tensor(m1e[:], m1[:], neyeP[:], Alu.mult)
                c_im = fe.tile([128, 1], f32, tag="cim", name="c_im")
                nc.vector.reduce_max(c_im[:], m1e[:], axis=X)

                nc.tensor.transpose(ps_t, sfull[:], eyef[:])
                de2 = fe.tile([128, 128], f32, tag="de2", name="de2")
                nc.vector.tensor_tensor(de2[:], ps_t, eyePT[:], Alu.mult)
                diag2 = fe.tile([128, 1], f32, tag="diag2", name="diag2")
                nc.vector.reduce_sum(diag2[:], de2[:], axis=X)
                dm2 = fe.tile([128, 1], f32, tag="dm2", name="dm2")
                nc.vector.tensor_scalar(dm2[:], diag2[:], 1.2, None,
                                        Alu.subtract)
                m2 = fe.tile([128, 128], f32, tag="m2", name="m2")
                nc.vector.tensor_scalar(m2[:], ps_t, dm2[:], 0.0,
                                        Alu.subtract, Alu.max)
                m2e = fe.tile([128, 128], f32, tag="m2e", name="m2e")
                nc.vector.tensor_tensor(m2e[:], m2[:], neyePT[:], Alu.mult)
                c_s = fe.tile([128, 1], f32, tag="cs", name="c_s")
                nc.vector.reduce_max(c_s[:], m2e[:], axis=X)

                tt = fe.tile([128, 1], f32, tag="tt", name="tt")
                nc.vector.tensor_tensor(tt[:], c_im[:], c_s[:], Alu.add)
                nc.tensor.matmul(ps_l, tt[:], onescol[:],
                                 start=True, stop=True)
                lsb = fe.tile([1, 1], f32, tag="lsb", name="lsb")
                nc.scalar.mul(lsb[:], ps_l, 1.0 / 6.0)
                if rep == reps - 1:
                    nc.sync.dma_start(out=loss_d[:], in_=lsb[:])

            # ---- flat pipeline driver --------------------------------------
            def ok(h):
                return 0 <= h < H

            # Stage->iteration mapping (global group h):
            #   s0,s1_act@h  s1_dve,s2@h+1  s_a2,s3,s_a2b@h+2  s4,s5@h+3
            #   s6,s7@h+4  s8,s9@h+5  s10@h+6
            # rep boundaries: cn2(0)@0, cn2(r)@r*NG+16; head(r)@(r+1)*NG+6
            # (before the next rep's first ps_neq write); tail(r) at +14 so
            # the AllGather has ~8 iterations to complete under compute.
            for it in range(H + 21):
                if it == 0:
                    emit_cn2(0)
                if it >= NG + 6 and (it - 6) % NG == 0:
                    r = (it - 6) // NG - 1
                    if r < reps:
                        emit_head_a(r)
                if it >= NG + 8 and (it - 8) % NG == 0:
                    r = (it - 8) // NG - 1
                    if r < reps:
                        emit_head_b(r)
                if ok(it - 3):
                    s4(it - 3)       # PE: sel matmuls
                if ok(it - 4):
                    s6(it - 4)       # PE: rinv expand
                if ok(it - 5):
                    s8(it - 5)       # PE: GE
                if ok(it - 6):
                    s10(it - 6)      # PE: ind block-sum
                if ok(it):
                    s0(it)           # PE: gram+attn
                if ok(it - 1):
                    s2(it - 1)       # PE: B/B^2 transposes
                if ok(it - 1):
                    s1_dve(it - 1)   # DVE: g_sb
                if ok(it - 2):
                    s_a2(it - 2)     # Pool: t10 = 10*B
                if ok(it - 2):
                    s3(it - 2)       # B2t: Act square / DVE copy
                if ok(it - 2):
                    s_a2b(it - 2)    # DVE: A2 = min(t10, B)
                if ok(it - 3):
                    s5(it - 3)       # Act: ln, exp
                if ok(it - 4):
                    s7_dve(it - 4)   # DVE: Bn
                if ok(it - 4):
                    s7_act(it - 4)   # Act: E
                if ok(it - 4):
                    s7_pa(it - 4)    # DVE: pair A-half
                if ok(it - 5):
                    s9(it - 5)       # DVE: pair Q-half
                if ok(it):
                    s1_act(it)       # Act: B
                if ok(it) and it % 2:
                    s2_dve(it)       # DVE: B^2 (odd groups)
                if with_loss and it >= NG + 14 and (it - 14) % NG == 0:
                    r = (it - 14) // NG - 1
                    if r < reps:
                        emit_tail(r)
                if it >= 16 and (it - 16) % NG == 0:
                    r = (it - 16) // NG + 1
                    if r < reps:
                        emit_cn2(r)

    # Pin activation-table selection to the one set that contains every
    # scalar-engine function we use (prelu, ln, exp, square, copy):
    # otherwise the inserter alternates sets and pays a 1.3us table load
    # per switch.
    from concourse import bacc as _bacc_mod
    _orig_tables = _bacc_mod.get_activation_tables

    def _pinned_tables(arch):
        t = _orig_tables(arch)
        keep = "natural_log_exp_and_others"
        return {k: (v if k == keep else set()) for k, v in t.items()}

    _bacc_mod.get_activation_tables = _pinned_tables
    try:
        nc.compile()
    finally:
        _bacc_mod.get_activation_tables = _orig_tables
    return nc


def _prep_in_maps(images, captions, cap_lens):
    bf16 = _bf16()
    images = np.ascontiguousarray(images, dtype=np.float32)
    captions = np.ascontiguousarray(captions, dtype=np.float32)
    cl = np.asarray(cap_lens, dtype=np.int64)

    bins, CWP = _plan(cl)
    CH = [0, CWP - 128]
    GC = GR + CWP

    imt = images.transpose(2, 0, 1).reshape(D, I * R)
    imt_p = np.concatenate([imt, imt[:, : IRP - I * R]], axis=1)
    imt_bf = np.ascontiguousarray(imt_p).astype(bf16)          # [D, IRP]

    inds = np.zeros((GR, NG * 128), dtype=bf16)
    for g in range(NG):
        for k in range(GR):
            m = GI * g + k // R
            if m < I:
                inds[k, g * 128 + m] = 1.0

    bmask = np.zeros((GR, GR), dtype=np.float32)
    for b in range(GI):
        bmask[b * R:(b + 1) * R, b * R:(b + 1) * R] = 1.0

    onesb = np.ones((128, 128), dtype=bf16)
    eyeb = np.eye(128, dtype=np.float32).astype(bf16)
    eyef = np.eye(128, dtype=np.float32)
    onescol = np.ones((128, 1), dtype=np.float32)

    # permutation-aware eye masks for the gathered [position, image] matrix
    P = np.concatenate([np.asarray(b) for b in bins])          # pos -> caption
    eyeP = np.zeros((128, 128), dtype=np.float32)
    eyeP[np.arange(128), P] = 1.0
    neyeP = (1.0 - eyeP).astype(np.float32)
    eyePT = np.ascontiguousarray(eyeP.T)
    neyePT = np.ascontiguousarray(neyeP.T)

    in_maps = []
    for r in range(NCORES):
        caps = bins[r]
        capP = np.zeros((D, CWP), dtype=np.float32)
        colslot = np.full(CWP, -1, dtype=np.int64)
        col = 0
        for s, c in enumerate(caps):
            L = int(cl[c])
            capP[:, col:col + L] = captions[c, :L, :].T
            colslot[col:col + L] = s
            col += L
        capP_bf = capP.astype(bf16)                            # [D, CWP]
        # interleave [imT_g | capT] per group so gram+attn is one matmul
        imcap = np.empty((D, NG * GC), dtype=bf16)
        for g in range(NG):
            imcap[:, g * GC:g * GC + GR] = \
                imt_bf[:, g * GR:(g + 1) * GR]
            imcap[:, g * GC + GR:(g + 1) * GC] = capP_bf
        imcap = imcap.reshape(NK, 128, NG * GC)
        padqc = np.broadcast_to(
            (colslot < 0).astype(np.float32)[None, :], (128, CWP)
        ).copy()

        sel = np.zeros((128, 2 * CS), dtype=bf16)
        expand = np.zeros((CS, CWP), dtype=bf16)
        for j in range(CWP):
            s = colslot[j]
            if s < 0:
                continue
            expand[s, j] = 1.0
            p = 0 if j < 128 else 1       # owner chunk (chunks overlap)
            sel[j - CH[p], p * CS + s] = 1.0

        in_maps.append({
            "imcap": imcap,
            "sel": sel,
            "expand": expand,
            "inds": inds,
            "bmask": bmask,
            "padqc": padqc,
            "onesb": onesb,
            "eyeb": eyeb,
            "eyef": eyef,
            "eyeP": eyeP,
            "neyeP": neyeP,
            "eyePT": eyePT,
            "neyePT": neyePT,
            "onescol": onescol,
        })
    return in_maps, CWP


def _get_nc(cwp: int, reps: int = 1, with_loss: bool = True):
    key = (cwp, reps, with_loss)
    if key not in _CACHE:
        _CACHE[key] = _build_program(cwp, reps, with_loss)
    return _CACHE[key]


def kernel(images, captions, cap_lens):
    from concourse.bass_utils import run_bass_kernel_spmd

    in_maps, cwp = _prep_in_maps(images, captions, cap_lens)
    nc = _get_nc(cwp)
    res = run_bass_kernel_spmd(nc, in_maps, core_ids=list(range(NCORES)))
    out = res.results[0]["loss"]
    return np.float32(np.asarray(out).reshape(()))
